# revision 1
# baseline (speedup 1.0000x reference)
"""Causal multi-head self-attention on 8 TRN2 NeuronCores.

Sharding: batch (2) x head-groups (4) -> 8 cores. Each core computes the
qkv projection for its 4 heads of its batch, full causal attention for
those heads, and a partial output projection (its head slice of w_out);
the host sums the 4 partials per batch.

Per-core pipeline (matmuls in float32r: 13-bit-mantissa fp32, 1 cyc/row):
  A) x -> x^T via PE transposes; Q^T,K^T (head dims on partitions) and
     V natural (with a ones column appended per head) via matmuls
     against host-pre-transposed weights.
  B) per (head, 512-wide q tile): S^T = K^T.T @ Q^T (k on partitions),
     P^T = exp(S^T/8) via ACT, staircase causal mask on diagonal
     blocks, O^T += [V|1].T @ P^T accumulated in PSUM — the ones column
     of V makes row 64 the softmax denominator. Normalize with DVE
     reciprocal + PE broadcast of 1/denom.
  C) partial[s, :] = sum_h aoT_h.T @ woT_h -> DRAM.
"""

import math
import numpy as np

import concourse.bacc as bacc
import concourse.mybir as mybir
import concourse.tile as tile
from concourse.masks import make_identity
from concourse.bass_utils import run_bass_kernel_spmd

F32 = mybir.dt.float32
F32R = mybir.dt.float32r
EXP = mybir.ActivationFunctionType.Exp

D_MODEL = 1024
HEAD_DIM = 64
B, S = 2, 2048
N_CORES = 8
OLOC = 256                  # 4 heads x 64 dims per core
SCALE = 1.0 / math.sqrt(HEAD_DIM)

QT = 512                    # q tile (free dim of S^T / O^T)
NQT = S // QT
KB = 128                    # k block (partitions of S^T)
SB = 256                    # s tile in projection phase A

_CACHE = {}


def build_nc():
    nc = bacc.Bacc("TRN2", target_bir_lowering=False, debug=False)

    x_d = nc.dram_tensor("x", [S, D_MODEL], F32, kind="ExternalInput")
    wqk_d = nc.dram_tensor("wqk_t", [D_MODEL, 512], F32R, kind="ExternalInput")
    wv_d = nc.dram_tensor("wv_t", [D_MODEL, OLOC], F32R, kind="ExternalInput")
    wo_d = nc.dram_tensor("wo_t", [OLOC, D_MODEL], F32R, kind="ExternalInput")
    out_d = nc.dram_tensor("out", [S, D_MODEL], F32, kind="ExternalOutput")

    with tile.TileContext(nc) as tc:
        with (
            tc.tile_pool(name="persist", bufs=1) as pp,
            tc.tile_pool(name="work", bufs=2) as wp,
            tc.tile_pool(name="psum", bufs=1, space="PSUM") as psp,
        ):
            ident = pp.tile([128, 128], F32)
            make_identity(nc, ident[:])

            # staircase causal mask: M[p, c] = 1 if p <= c - 384 else 0.
            # Slice [:, (3-j)*128 :][:512] masks diagonal subblock j.
            mask = pp.tile([128, 896], F32)
            nc.gpsimd.memset(mask[:], 1.0)
            nc.gpsimd.affine_select(
                out=mask[:], in_=mask[:],
                compare_op=mybir.AluOpType.is_ge,
                fill=0.0, base=-384,
                pattern=[[1, 896]], channel_multiplier=-1,
            )

            ones_f = pp.tile([1, 64], F32)
            nc.gpsimd.memset(ones_f[:], 1.0)
            ones_r = pp.tile([1, 64], F32R)
            nc.vector.tensor_copy(ones_r[:], ones_f[:])
            ones4 = pp.tile([128, 4, 1], F32)
            nc.gpsimd.memset(ones4[:], 1.0)

            # weights (pre-transposed on host)
            wqk = [pp.tile([128, 512], F32R, name=f"wqk{i}") for i in range(8)]
            wv = [pp.tile([128, OLOC], F32R, name=f"wv{i}") for i in range(8)]
            for i in range(8):
                nc.sync.dma_start(wqk[i][:], wqk_d[i * 128:(i + 1) * 128, :])
                nc.sync.dma_start(wv[i][:], wv_d[i * 128:(i + 1) * 128, :])
            wo = [pp.tile([64, D_MODEL], F32R, name=f"wo{h}") for h in range(4)]
            for h in range(4):
                nc.sync.dma_start(wo[h][:], wo_d[h * 64:(h + 1) * 64, :])

            # persistent activations
            qkT = [pp.tile([128, S], F32R, name=f"qkT{ob}") for ob in range(4)]
            v_sb = [pp.tile([128, 4 * 65], F32R, name=f"v{j}")
                    for j in range(S // 128)]
            aoT = [pp.tile([64, S], F32R, name=f"aoT{h}") for h in range(4)]

            # ---- Phase A: x^T, Q^T/K^T, V ----
            for sb in range(S // SB):
                xn = wp.tile([128, 2, D_MODEL], F32, tag="xn", bufs=2)
                for j in range(2):
                    nc.sync.dma_start(
                        xn[:, j, :],
                        x_d[sb * SB + j * 128:sb * SB + (j + 1) * 128, :])
                xT = wp.tile([128, 8, SB], F32R, tag="xT", bufs=2)
                for it in range(8):
                    pt = psp.tile([128, SB], F32, tag="mm", bufs=2)
                    for j in range(2):
                        nc.tensor.matmul(
                            pt[:, j * 128:(j + 1) * 128],
                            xn[:, j, it * 128:(it + 1) * 128],
                            ident[:], is_transpose=True,
                            start=True, stop=True)
                    nc.vector.tensor_copy(xT[:, it, :], pt[:])
                # Q^T / K^T: psum (128 o, SB s) accumulated over 8 i-tiles
                for ob in range(4):
                    pqk = psp.tile([128, SB], F32, tag="mm", bufs=2)
                    for it in range(8):
                        nc.tensor.matmul(
                            pqk[:],
                            wqk[it][:, ob * 128:(ob + 1) * 128],
                            xT[:, it, :],
                            start=(it == 0), stop=(it == 7))
                    nc.vector.tensor_copy(qkT[ob][:, sb * SB:(sb + 1) * SB], pqk[:])
                # V natural per 128-row s block, interleaved [V_h | 1]
                for j in range(2):
                    pv = psp.tile([128, OLOC], F32, tag="mm", bufs=2)
                    for it in range(8):
                        nc.tensor.matmul(
                            pv[:],
                            xT[:, it, j * 128:(j + 1) * 128],
                            wv[it][:],
                            start=(it == 0), stop=(it == 7))
                    vt = v_sb[sb * 2 + j]
                    vt3 = vt.rearrange("p (h d) -> p h d", h=4)
                    nc.vector.tensor_copy(vt3[:, :, 64:65], ones4[:])
                    nc.vector.tensor_copy(
                        vt3[:, :, 0:64],
                        pv[:].rearrange("p (h d) -> p h d", h=4))

            # ---- Phase B: attention ----
            for h in range(4):
                q_ap = qkT[h // 2][64 * (h % 2):64 * (h % 2) + 64, :]
                k_ap = qkT[2 + h // 2][64 * (h % 2):64 * (h % 2) + 64, :]
                for qt in range(NQT):
                    po = psp.tile([65, QT], F32, tag="po", bufs=2)
                    nkb = (qt + 1) * (QT // KB)   # 4, 8, 12, 16
                    for grp in range(nkb // 2):
                        pst = psp.tile([128, 1024], F32, tag="pst", bufs=1)
                        for u in range(2):
                            kb = grp * 2 + u
                            nc.tensor.matmul(
                                pst[:, u * 512:(u + 1) * 512],
                                k_ap[:, kb * KB:(kb + 1) * KB],
                                q_ap[:, qt * QT:(qt + 1) * QT],
                                start=True, stop=True)
                        p_t = wp.tile([128, 1024], F32R, tag="p_t", bufs=3)
                        nc.scalar.activation(p_t[:], pst[:], EXP, scale=SCALE)
                        for u in range(2):
                            kb = grp * 2 + u
                            j = kb - (nkb - 4)
                            if j >= 0:  # diagonal band: staircase mask
                                nc.vector.tensor_mul(
                                    p_t[:, u * 512:(u + 1) * 512],
                                    p_t[:, u * 512:(u + 1) * 512],
                                    mask[:, (3 - j) * 128:(3 - j) * 128 + 512])
                            nc.tensor.matmul(
                                po[:],
                                v_sb[kb][:, h * 65:(h + 1) * 65],
                                p_t[:, u * 512:(u + 1) * 512],
                                start=(kb == 0), stop=(kb == nkb - 1),
                                skip_group_check=True)
                    # normalize: 1/denom, broadcast via PE, multiply
                    with nc.allow_low_precision(reason="f32r recip"):
                        recip = wp.tile([1, QT], F32R, tag="recip", bufs=2)
                        nc.vector.reciprocal(recip[:], po[64:65, :])
                    pbc = psp.tile([64, QT], F32, tag="pbc", bufs=1)
                    nc.tensor.matmul(pbc[:], ones_r[:], recip[:],
                                     start=True, stop=True)
                    rbc = wp.tile([64, QT], F32, tag="rbc", bufs=2)
                    nc.scalar.copy(rbc[:], pbc[:])
                    nc.vector.tensor_mul(
                        aoT[h][:, qt * QT:(qt + 1) * QT], po[0:64, :], rbc[:])

            # ---- Phase C: output projection (partial) ----
            for sb2 in range(S // 128):
                for ob in range(2):
                    pout = psp.tile([128, 512], F32, tag="mm", bufs=2)
                    for h in range(4):
                        nc.tensor.matmul(
                            pout[:],
                            aoT[h][:, sb2 * 128:(sb2 + 1) * 128],
                            wo[h][:, ob * 512:(ob + 1) * 512],
                            start=(h == 0), stop=(h == 3))
                    osb = wp.tile([128, 512], F32, tag="osb", bufs=3)
                    nc.vector.tensor_copy(osb[:], pout[:])
                    nc.sync.dma_start(
                        out_d[sb2 * 128:(sb2 + 1) * 128, ob * 512:(ob + 1) * 512],
                        osb[:])

    nc.compile()
    return nc


def make_in_maps(x, w_qkv, w_out):
    in_maps = []
    for c in range(N_CORES):
        b, g = divmod(c, 4)
        wq = w_qkv[g * OLOC:(g + 1) * OLOC, :]
        wk = w_qkv[D_MODEL + g * OLOC:D_MODEL + (g + 1) * OLOC, :]
        wv = w_qkv[2 * D_MODEL + g * OLOC:2 * D_MODEL + (g + 1) * OLOC, :]
        in_maps.append({
            "x": np.ascontiguousarray(x[b]),
            "wqk_t": np.ascontiguousarray(np.concatenate([wq, wk], axis=0).T),
            "wv_t": np.ascontiguousarray(wv.T),
            "wo_t": np.ascontiguousarray(w_out[:, g * OLOC:(g + 1) * OLOC].T),
        })
    return in_maps


def kernel(x, w_qkv, w_out):
    x = np.asarray(x, dtype=np.float32)
    w_qkv = np.asarray(w_qkv, dtype=np.float32)
    w_out = np.asarray(w_out, dtype=np.float32)

    if "nc" not in _CACHE:
        _CACHE["nc"] = build_nc()
    nc = _CACHE["nc"]

    in_maps = make_in_maps(x, w_qkv, w_out)
    _CACHE["in_maps"] = in_maps

    res = run_bass_kernel_spmd(nc, in_maps, list(range(N_CORES)))
    out = np.zeros((B, S, D_MODEL), dtype=np.float32)
    for c in range(N_CORES):
        out[c // 4] += res.results[c]["out"]
    return out



# revision 6
# speedup vs baseline: 1.9165x; 1.9165x over previous
"""Causal multi-head self-attention on 8 TRN2 NeuronCores.

Sharding: batch (2) x head-groups (4) -> 8 cores. Each core computes the
qkv projection for its 4 heads of its batch, full causal attention for
those heads, and a partial output projection (its head slice of w_out);
the host sums the 4 partials per batch.

Per-core pipeline (v3):
  A) x^T arrives pre-transposed (bf16) from the host, so no PE
     transposes. Q^T/K^T (f32r in SBUF, head dims on partitions) via
     w^T-stationary bf16 matmuls; V natural ([s, d]) bf16 with a ones
     column per head (softmax denominator rides the PV matmul).
  B) attention in 256-wide q tiles: S^T = K^T.T @ Q^T (k on partitions,
     f32r), P = exp(S/8) on ACT into bf16, staircase mask on the two
     diagonal k-blocks (DVE), then O[q, d] += P_slice.T @ [V|1] with P as
     the 128-wide bf16 stationary. O is q-on-partitions, so the softmax
     denominator is a per-partition scalar: DVE reciprocal +
     tensor_scalar_mul, fused with the PSUM->SBUF bf16 copy. Two heads
     pack into one [128, 128] bf16 tile, PE-transposed into
     aoT2[hp] = [2 heads' dims, s].
  C) partial[s, :] = sum_hp aoT2[hp].T @ wo2[hp], staged bf16 and DMA'd;
     host converts/sums partials in f32.

Emission is software-pipelined and deficit-paced: each group's PV is one
unit late (PE runs ahead of ACT's exp); projection chains for s-tile
st+1, deferred O-transposes, and output-projection chunks are queued as
PE fillers and dispensed whenever the running ACT-vs-PE budget goes
negative, so PE never starves while ACT crunches exp. Leftover fillers
(mostly phase C) drain at the end, covering ACT's causal-tail overrun.
"""

import math
import numpy as np

import concourse.bacc as bacc
import concourse.mybir as mybir
import concourse.tile as tile
from concourse.masks import make_identity
from concourse.bass_utils import run_bass_kernel_spmd

F32 = mybir.dt.float32
F32R = mybir.dt.float32r
BF16 = mybir.dt.bfloat16
EXP = mybir.ActivationFunctionType.Exp

D_MODEL = 1024
HEAD_DIM = 64
B, S = 2, 2048
N_CORES = 8
OLOC = 256                  # 4 heads x 64 dims per core
SCALE = 1.0 / math.sqrt(HEAD_DIM)
G = 4                       # k-blocks (128 each) per S/exp group

PE_NS = 1.0 / 2.4           # ns per PE cycle at full p-state
ACT_NS = 1.0 / 1.2

_CACHE = {}


def build_nc():
    nc = bacc.Bacc("TRN2", target_bir_lowering=False, debug=False)

    x_d = nc.dram_tensor("x_t", [D_MODEL, S], BF16, kind="ExternalInput")
    wqk_d = nc.dram_tensor("wqk_t", [D_MODEL, 512], BF16, kind="ExternalInput")
    wv_d = nc.dram_tensor("wv_t", [D_MODEL, OLOC], BF16, kind="ExternalInput")
    wo_d = nc.dram_tensor("wo_t", [OLOC, D_MODEL], BF16, kind="ExternalInput")
    out_d = nc.dram_tensor("out", [S, D_MODEL], BF16, kind="ExternalOutput")

    with tile.TileContext(nc) as tc:
        with (
            tc.tile_pool(name="persist", bufs=1) as pp,
            tc.tile_pool(name="work", bufs=2) as wp,
            tc.tile_pool(name="psum", bufs=1, space="PSUM") as psp,
        ):
            ident = pp.tile([128, 128], BF16)
            make_identity(nc, ident[:])

            # staircase causal mask for the 2-block diagonal band of a
            # 256-wide q tile: M[p, c] = 1 iff p <= c - 128. Slice
            # [:, (1-j)*128:][:256] masks diagonal sub-block j.
            mask = pp.tile([128, 384], BF16)
            nc.gpsimd.memset(mask[:], 1.0)
            nc.gpsimd.affine_select(
                out=mask[:], in_=mask[:],
                compare_op=mybir.AluOpType.is_ge,
                fill=0.0, base=-128,
                pattern=[[1, 384]], channel_multiplier=-1,
            )

            xT = pp.tile([128, 8, S], BF16)
            wqk = pp.tile([128, 8, 512], BF16)
            wv = pp.tile([128, 8, OLOC], BF16)
            wo2 = pp.tile([128, 2, D_MODEL], BF16)
            qkT = [pp.tile([128, S], F32R, name=f"qkT{i}") for i in range(4)]
            v_sb = [pp.tile([128, 4, 65], BF16, name=f"v{j}")
                    for j in range(S // 128)]
            aoT2 = [pp.tile([128, S], BF16, name=f"aoT{i}") for i in range(2)]

            for j in range(S // 128):
                nc.gpsimd.memset(v_sb[j][:, :, 64:65], 1.0)

            # input DMAs (all SP queue), ordered so the first s-tile's
            # operands land first: wqk halves + st0 x slices, then wv,
            # then coarser x chunks for s 512.. plus wo.
            nc.sync.dma_start(
                wqk[:, 0:4, :],
                wqk_d[0:512, :].rearrange("(c p) o -> p c o", p=128))
            for it in range(4):
                nc.sync.dma_start(
                    xT[:, it, 0:512],
                    x_d[it * 128:(it + 1) * 128, 0:512])
            nc.sync.dma_start(
                wqk[:, 4:8, :],
                wqk_d[512:1024, :].rearrange("(c p) o -> p c o", p=128))
            for it in range(4, 8):
                nc.sync.dma_start(
                    xT[:, it, 0:512],
                    x_d[it * 128:(it + 1) * 128, 0:512])
            nc.sync.dma_start(
                wv[:],
                wv_d[:, :].rearrange("(c p) o -> p c o", p=128))
            for it in range(8):
                nc.sync.dma_start(
                    xT[:, it, 512:1280],
                    x_d[it * 128:(it + 1) * 128, 512:1280])
            nc.sync.dma_start(
                wo2[:],
                wo_d[:, :].rearrange("(hp p) o -> p hp o", p=128))
            for it in range(8):
                nc.sync.dma_start(
                    xT[:, it, 1280:2048],
                    x_d[it * 128:(it + 1) * 128, 1280:2048])

            # ---- pacing scheduler ----
            # bank = accumulated (PE work - ACT work) in ns over the
            # attention stream; when it goes negative, PE would starve
            # waiting on exp, so dispense queued PE filler work.
            sched = {"bank": 3000.0}
            proj_q = []          # projection chains (deadline: next pair)
            late_q = []          # transposes + phase C (no deadline)

            def dispense():
                while sched["bank"] < 0.0 and (proj_q or late_q):
                    q = proj_q if proj_q else late_q
                    pe_ns, fn = q.pop(0)
                    fn()
                    sched["bank"] += pe_ns

            def charge(pe_ns, act_ns):
                sched["bank"] += pe_ns - act_ns
                dispense()

            def flush_proj():
                while proj_q:
                    _, fn = proj_q.pop(0)
                    fn()

            # ---- phase A: Q^T/K^T + V for one 512-wide s tile ----
            def qk_chain(st, ob):
                pqk = psp.tile([128, 512], F32, tag="mm", bufs=2, name="pqk")
                for it in range(8):
                    nc.tensor.matmul(
                        pqk[:],
                        wqk[:, it, ob * 128:(ob + 1) * 128],
                        xT[:, it, st * 512:(st + 1) * 512],
                        start=(it == 0), stop=(it == 7),
                        skip_group_check=True)
                nc.vector.tensor_copy(
                    qkT[ob][:, st * 512:(st + 1) * 512], pqk[:])

            def v_chain(st, j):
                pv = psp.tile([128, OLOC], F32, tag="mm", bufs=2, name="pv")
                s0 = st * 512 + j * 128
                for it in range(8):
                    nc.tensor.matmul(
                        pv[:],
                        xT[:, it, s0:s0 + 128],
                        wv[:, it, :],
                        start=(it == 0), stop=(it == 7),
                        skip_group_check=True)
                vt = v_sb[st * 4 + j]
                nc.vector.tensor_copy(
                    vt[:, :, 0:64],
                    pv[:].rearrange("p (h d) -> p h d", h=4))

            def queue_phaseA(st):
                for ob in range(4):
                    proj_q.append((8 * 512 * PE_NS,
                                   lambda st=st, ob=ob: qk_chain(st, ob)))
                for j in range(4):
                    proj_q.append((8 * 256 * PE_NS,
                                   lambda st=st, j=j: v_chain(st, j)))

            # ---- phase C: one 128-row output chunk ----
            osb_tiles = {}

            def c_chunk(qt, sb, ob):
                s0 = qt * 256 + sb * 128
                pout = psp.tile([128, 512], F32, tag="mm", bufs=2,
                                name="pout")
                for hp in range(2):
                    nc.tensor.matmul(
                        pout[:],
                        aoT2[hp][:, s0:s0 + 128],
                        wo2[:, hp, ob * 512:(ob + 1) * 512],
                        start=(hp == 0), stop=(hp == 1),
                        skip_group_check=True)
                if ob == 0:
                    osb_tiles[qt, sb] = wp.tile([128, 2, 512], BF16,
                                                tag="osb", bufs=3,
                                                name="osb")
                osb = osb_tiles[qt, sb]
                nc.vector.tensor_copy(osb[:, ob, :], pout[:])
                if ob == 1:
                    nc.sync.dma_start(out_d[s0:s0 + 128, :],
                                      osb_tiles.pop((qt, sb))[:])

            def queue_phaseC(qt):
                for sb in range(2):
                    for ob in range(2):
                        late_q.append(
                            (2 * 512 * PE_NS,
                             lambda qt=qt, sb=sb, ob=ob: c_chunk(qt, sb, ob)))

            # ---- phase B: attention with lag-1 PV emission ----
            pending = [None]

            def run_unit(s_fn, pv_fn, posts, pe_ns, act_ns):
                s_fn()
                prev = pending[0]
                pending[0] = (pv_fn, posts)
                if prev is not None:
                    prev[0]()
                    for p in prev[1]:
                        p()
                charge(pe_ns, act_ns)

            def flush_pending():
                prev = pending[0]
                pending[0] = None
                if prev is not None:
                    prev[0]()
                    for p in prev[1]:
                        p()

            o2_tiles = {}

            def emit_head(qt, hp, hh):
                h = 2 * hp + hh
                nkb = 2 * (qt + 1)
                r0 = (h % 2) * 64
                q_t = qkT[h // 2]
                k_t = qkT[2 + h // 2]
                state = {}
                ngrp = (nkb + G - 1) // G

                def s_fn(kb0, g):
                    pst = psp.tile([128, G, 256], F32, tag="pst", bufs=2,
                                   name="pst")
                    for u in range(g):
                        kb = kb0 + u
                        nc.tensor.matmul(
                            pst[:, u, :],
                            k_t[r0:r0 + 64, kb * 128:(kb + 1) * 128],
                            q_t[r0:r0 + 64, qt * 256:(qt + 1) * 256],
                            start=True, stop=True)
                    p_t = wp.tile([128, G, 256], BF16, tag="p_t", bufs=3,
                                  name="p_t")
                    nc.scalar.activation(p_t[:, 0:g, :], pst[:, 0:g, :],
                                         EXP, scale=SCALE)
                    for u in range(g):
                        j = kb0 + u - (nkb - 2)
                        if j >= 0:  # diagonal band: staircase mask
                            nc.vector.tensor_mul(
                                p_t[:, u, :], p_t[:, u, :],
                                mask[:, (1 - j) * 128:(1 - j) * 128 + 256])
                    state["p_t", kb0] = p_t

                def pv_fn(kb0, g):
                    if kb0 == 0:
                        # one PSUM bank per open accumulation group: a
                        # second group's start in the same bank wipes the
                        # first group's partials
                        state["po"] = [
                            psp.tile([128, 65], F32, tag="po", bufs=2,
                                     name="po")
                            for _ in range(2)]
                    po = state["po"]
                    p_t = state.pop(("p_t", kb0))
                    for u in range(g):
                        kb = kb0 + u
                        for q2 in range(2):
                            nc.tensor.matmul(
                                po[q2][:],
                                p_t[:, u, q2 * 128:(q2 + 1) * 128],
                                v_sb[kb][:, h, :],
                                start=(kb == 0), stop=(kb == nkb - 1),
                                skip_group_check=True)

                def norm_fn():
                    po = state["po"]
                    if hh == 0:
                        o2_tiles[qt, hp] = [
                            wp.tile([128, 128], BF16, tag="o2", bufs=8,
                                    name="o2")
                            for _ in range(2)]
                    o2 = o2_tiles[qt, hp]
                    for q2 in range(2):
                        recip = wp.tile([128, 1], F32, tag="recip", bufs=2,
                                        name="recip")
                        nc.vector.reciprocal(recip[:], po[q2][:, 64:65])
                        nc.vector.tensor_scalar_mul(
                            o2[q2][:, hh * 64:(hh + 1) * 64],
                            po[q2][:, 0:64], recip[:])

                def trans_fn():
                    o2 = o2_tiles.pop((qt, hp))
                    for q2 in range(2):
                        ptr = psp.tile([128, 128], BF16, tag="mm", bufs=2,
                                       name="ptr")
                        nc.tensor.matmul(ptr[:], o2[q2][:], ident[:],
                                         is_transpose=True,
                                         skip_group_check=True)
                        nc.vector.tensor_copy(
                            aoT2[hp][:, qt * 256 + q2 * 128:
                                     qt * 256 + (q2 + 1) * 128], ptr[:])

                def queue_tail():
                    late_q.append((2 * 128 * PE_NS, trans_fn))
                    if hp == 1:
                        queue_phaseC(qt)

                prev_g = [0]
                for gi in range(ngrp):
                    kb0 = gi * G
                    g = min(G, nkb - kb0)
                    posts = []
                    if gi == ngrp - 1:
                        posts.append(norm_fn)
                        if hh == 1:
                            posts.append(queue_tail)
                    pe_ns = (256 * g + 130 * prev_g[0]) * PE_NS
                    act_ns = (256 * g + 222) * ACT_NS + 32
                    run_unit(lambda kb0=kb0, g=g: s_fn(kb0, g),
                             lambda kb0=kb0, g=g: pv_fn(kb0, g),
                             posts, pe_ns, act_ns)
                    prev_g[0] = g

            # ---- schedule ----
            for ob in range(4):
                qk_chain(0, ob)
            for j in range(4):
                v_chain(0, j)
            for st in range(4):
                if st < 3:
                    queue_phaseA(st + 1)
                for qt in (2 * st, 2 * st + 1):
                    for hp in range(2):
                        for hh in range(2):
                            emit_head(qt, hp, hh)
                flush_proj()
            flush_pending()
            while late_q:
                _, fn = late_q.pop(0)
                fn()

    nc.compile()
    return nc


def make_in_maps(x, w_qkv, w_out):
    import ml_dtypes
    bf = ml_dtypes.bfloat16
    in_maps = []
    for c in range(N_CORES):
        b, g = divmod(c, 4)
        wq = w_qkv[g * OLOC:(g + 1) * OLOC, :]
        wk = w_qkv[D_MODEL + g * OLOC:D_MODEL + (g + 1) * OLOC, :]
        wvs = w_qkv[2 * D_MODEL + g * OLOC:2 * D_MODEL + (g + 1) * OLOC, :]
        in_maps.append({
            "x_t": np.ascontiguousarray(x[b].T).astype(bf),
            "wqk_t": np.ascontiguousarray(
                np.concatenate([wq, wk], axis=0).T).astype(bf),
            "wv_t": np.ascontiguousarray(wvs.T).astype(bf),
            "wo_t": np.ascontiguousarray(
                w_out[:, g * OLOC:(g + 1) * OLOC].T).astype(bf),
        })
    return in_maps


def kernel(x, w_qkv, w_out):
    x = np.asarray(x, dtype=np.float32)
    w_qkv = np.asarray(w_qkv, dtype=np.float32)
    w_out = np.asarray(w_out, dtype=np.float32)

    if "nc" not in _CACHE:
        _CACHE["nc"] = build_nc()
    nc = _CACHE["nc"]

    in_maps = make_in_maps(x, w_qkv, w_out)
    _CACHE["in_maps"] = in_maps

    res = run_bass_kernel_spmd(nc, in_maps, list(range(N_CORES)))
    out = np.zeros((B, S, D_MODEL), dtype=np.float32)
    for c in range(N_CORES):
        out[c // 4] += np.asarray(res.results[c]["out"], dtype=np.float32)
    return out
